# revision 2
# baseline (speedup 1.0000x reference)
"""Trainium2 Bass kernel for nn_GSCAN_model (gnn_message_passing).

Reference computation (per cell of a [B, 32, 32, 17] grid):
    emb    = concat(x[0:4] @ W_size, x[4:8] @ W_shape,
                    x[8:12] @ W_rgb, x[12:17] @ W_agent)     # [64]
    mask   = sum(x) > 0
    out    = mask ? emb : [x, zeros(47)]                     # [64]

This is memory-bound, so the kernel is organized around minimizing HBM
traffic and keeping the DMA rings saturated.  v2 traffic diet (vs the
244us v1 that shipped xm+px bf16 and stored f32):
  - inputs:  raw x bf16 [cells,17] plus a per-cell mask m bf16 [cells]
             (36 B/cell instead of 68 B/cell for xm+px),
  - output:  y bf16 [cells,64] (128 B/cell instead of 256 B/cell),
             upconverted to f32 on the host.
Total 164 B/cell = 43 MB/core -> ~120us at the 358 GB/s per-core DMA
roofline (vs 85 MB/core = 237us for v1).

On-chip the mask is applied with a stride-0 broadcast multiply:
    xm = x * m[..., None]      (DVE, bf16)
    px = x - xm                (DVE, bf16; exact: xm is x or 0)
then the v1 pipeline: PE transposes batch 7 cell-slots, matmuls against
the block-diagonal Wd [119,448] land cells back on partitions, PSUM
drains cast f32->bf16 split DVE/ACT, GPSIMD adds the px passthrough
into out[:, :, 0:17], and each span's store launches as soon as its
drains+add complete.  Loads issue on the ACT HWDGE ring; stores split
across the GPSIMD and SP rings.

Layout: macro tiles of 128 partitions x 128 cells; per partition the
input run is 4352 B and the output run is 16 KiB contiguous.

Data parallel over 8 NeuronCores: batch dim 2048 -> 256 per core.
"""

import numpy as np
import ml_dtypes

B, H, W, C_IN = 2048, 32, 32, 17
EMB = 64
N_CORES = 8
P = 128                      # partitions
C_SLOTS = 128                # cells per partition per macro tile
CELLS_PER_CORE = (B // N_CORES) * H * W          # 262144
MACROS = CELLS_PER_CORE // (P * C_SLOTS)         # 16
# groups of cell-slots per macro: 18 groups of 7 slots + 1 group of 2
GROUPS = [(7 * i, 7) for i in range(18)] + [(126, 2)]
KW = 7 * C_IN                # 119 rows: largest weight-block group
NW = 7 * EMB                 # 448 cols
# px-passthrough adds, gated on spans of drained groups; the store is
# split the same way so each span's DMA launches as soon as its
# drains+add complete instead of waiting for the whole macro
ADD_SPANS = [(0, 0, 63), (9, 63, 128)]
V_DRAIN = {1, 3, 5, 7, 10, 12, 14, 16, 18}  # DVE's share of PSUM drains

_CACHE = {}


def _build_program(n_macros):
    import concourse.bacc as bacc
    import concourse.mybir as mybir
    from concourse.tile import TileContext

    f32 = mybir.dt.float32
    bf16 = mybir.dt.bfloat16
    nc = bacc.Bacc("TRN2", target_bir_lowering=False, debug=False,
                   num_devices=N_CORES)

    cells = n_macros * P * C_SLOTS
    x_d = nc.dram_tensor("x", [cells, C_IN], bf16, kind="ExternalInput")
    m_d = nc.dram_tensor("m", [cells], bf16, kind="ExternalInput")
    wd = nc.dram_tensor("wd", [KW, NW], bf16, kind="ExternalInput")
    ident = nc.dram_tensor("ident", [P, P], bf16, kind="ExternalInput")
    y = nc.dram_tensor("y", [cells, EMB], bf16, kind="ExternalOutput")

    xr = x_d.ap().rearrange("(m p c) k -> m p (c k)", p=P, c=C_SLOTS)
    mr = m_d.ap().rearrange("(m p c) -> m p c", p=P, c=C_SLOTS)
    yr = y.ap().rearrange("(m p c) n -> m p (c n)", p=P, c=C_SLOTS)

    OCTS = [GROUPS[q * 8:(q + 1) * 8] for q in range(3)]

    with TileContext(nc) as tc:
        with (
            tc.tile_pool(name="const", bufs=1) as constp,
            tc.tile_pool(name="xp", bufs=3) as x_pool,
            tc.tile_pool(name="mp", bufs=3) as m_pool,
            tc.tile_pool(name="xmp", bufs=2) as xm_pool,
            tc.tile_pool(name="pxp", bufs=3) as px_pool,
            tc.tile_pool(name="xat", bufs=2) as xat_pool,
            tc.tile_pool(name="outp", bufs=3) as out_pool,
            tc.tile_pool(name="pst", bufs=2, space="PSUM") as pst_pool,
            tc.tile_pool(name="pso", bufs=6, space="PSUM") as pso_pool,
        ):
            wd_t = constp.tile([KW, NW], bf16)
            nc.scalar.dma_start(out=wd_t, in_=wd.ap())
            id_t = constp.tile([P, P], bf16)
            nc.scalar.dma_start(out=id_t, in_=ident.ap())

            state = {}

            def load(mi):
                x = x_pool.tile([P, C_SLOTS * C_IN], bf16)
                nc.scalar.dma_start(out=x, in_=xr[mi])
                m = m_pool.tile([P, C_SLOTS], bf16)
                nc.scalar.dma_start(out=m, in_=mr[mi])
                state[mi] = {"x": x, "m": m}

            def mask_mul(mi):
                """xm = x * m (broadcast over the 17 channels)."""
                st = state[mi]
                xm = xm_pool.tile([P, C_SLOTS * C_IN], bf16)
                x3 = st["x"].rearrange("p (c k) -> p c k", k=C_IN)
                xm3 = xm.rearrange("p (c k) -> p c k", k=C_IN)
                mb = st["m"].unsqueeze(2).broadcast_to([P, C_SLOTS, C_IN])
                nc.vector.tensor_tensor(out=xm3, in0=x3, in1=mb,
                                        op=mybir.AluOpType.mult)
                st["xm"] = xm

            def px_sub(mi):
                """px = x - xm (exact in bf16: xm is x or 0)."""
                st = state[mi]
                px = px_pool.tile([P, C_SLOTS * C_IN], bf16)
                nc.vector.tensor_tensor(out=px, in0=st["x"], in1=st["xm"],
                                        op=mybir.AluOpType.subtract)
                st["px"] = px

            def front(mi):
                """PE transposes + matmuls for macro mi."""
                st = state[mi]
                xm = st["xm"]
                tps = []
                for oct_ in OCTS:
                    tp = pst_pool.tile([P, 8 * P], bf16, tag="tp")
                    for j, (c0, ns) in enumerate(oct_):
                        k = ns * C_IN
                        nc.tensor.transpose(
                            out=tp[0:k, j * P:(j + 1) * P],
                            in_=xm[:, c0 * C_IN:(c0 + ns) * C_IN],
                            identity=id_t)
                    tps.append(tp)
                xat = xat_pool.tile([P, len(GROUPS) * P], bf16)
                for gi, (c0, ns) in enumerate(GROUPS):
                    k = ns * C_IN
                    src = tps[gi // 8][0:k, (gi % 8) * P:(gi % 8 + 1) * P]
                    nc.vector.tensor_copy(out=xat[0:k, gi * P:(gi + 1) * P],
                                          in_=src)
                pos = []
                for gi, (c0, ns) in enumerate(GROUPS):
                    k = ns * C_IN
                    n = ns * EMB
                    po = pso_pool.tile([P, NW], f32, tag="po")
                    nc.tensor.matmul(out=po[:, 0:n],
                                     lhsT=xat[0:k, gi * P:(gi + 1) * P],
                                     rhs=wd_t[0:k, 0:n],
                                     start=True, stop=True)
                    pos.append(po)
                st["pos"] = pos

            def drain(mi):
                """PSUM drain (f32->bf16) + px passthrough + store."""
                st = state.pop(mi)
                pos = st["pos"]
                px3 = st["px"].rearrange("p (c k) -> p c k", k=C_IN)
                out_t = out_pool.tile([P, C_SLOTS * EMB], bf16)
                out3 = out_t.rearrange("p (c n) -> p c n", n=EMB)
                span_g1 = [g for g, _, _ in ADD_SPANS[1:]] + [len(GROUPS)]
                for si, (g0, a0, a1) in enumerate(ADD_SPANS):
                    g1 = span_g1[si]
                    for gi in range(g0, g1):
                        c0, ns = GROUPS[gi]
                        n = ns * EMB
                        dst = out_t[:, c0 * EMB:c0 * EMB + n]
                        if gi in V_DRAIN:
                            nc.vector.tensor_copy(out=dst,
                                                  in_=pos[gi][:, 0:n])
                        else:
                            nc.scalar.copy(out=dst, in_=pos[gi][:, 0:n])
                    nc.gpsimd.tensor_tensor(
                        out=out3[:, a0:a1, 0:C_IN],
                        in0=out3[:, a0:a1, 0:C_IN],
                        in1=px3[:, a0:a1, :],
                        op=mybir.AluOpType.add)
                    # stores split across the GPSIMD and SP HWDGE rings;
                    # loads go on ACT
                    eng = nc.gpsimd if si == 0 else nc.sync
                    eng.dma_start(
                        out=yr[mi][:, a0 * EMB:a1 * EMB],
                        in_=out_t[:, a0 * EMB:a1 * EMB])

            # software pipeline: loads lead by one macro; macro m's drain
            # is emitted one iteration behind its matmuls, and DVE's
            # drain copies precede its xat copies so the PSUM po-buffer
            # rotation never blocks ready work behind not-ready work.
            load(0)
            for mi in range(n_macros + 1):
                if mi + 1 < n_macros:
                    load(mi + 1)
                if mi < n_macros:
                    mask_mul(mi)
                if mi >= 1:
                    drain(mi - 1)
                if mi < n_macros:
                    front(mi)
                    px_sub(mi)
    nc.compile()
    return nc


def _host_weights(W_size, W_shape, W_rgb, W_agent):
    """Wd [119, 448] bf16: 7 diagonal blocks of the assembled Wblk."""
    wblk = np.zeros((C_IN, EMB), np.float32)
    wblk[0:4, 0:16] = W_size
    wblk[4:8, 16:32] = W_shape
    wblk[8:12, 32:48] = W_rgb
    wblk[12:17, 48:64] = W_agent
    wd = np.zeros((KW, NW), np.float32)
    for i in range(7):
        wd[i * C_IN:(i + 1) * C_IN, i * EMB:(i + 1) * EMB] = wblk
    return wd.astype(ml_dtypes.bfloat16)


def _in_maps(situation, W_size, W_shape, W_rgb, W_agent):
    wd = _host_weights(np.asarray(W_size, np.float32),
                       np.asarray(W_shape, np.float32),
                       np.asarray(W_rgb, np.float32),
                       np.asarray(W_agent, np.float32))
    ident = np.eye(P, dtype=ml_dtypes.bfloat16)
    sit = np.ascontiguousarray(np.asarray(situation), dtype=np.float32)
    mask = sit.sum(axis=-1) > 0                       # [B, H, W]
    x_full = sit.astype(ml_dtypes.bfloat16)
    m_full = mask.astype(ml_dtypes.bfloat16)
    bpc = B // N_CORES
    in_maps = []
    for i in range(N_CORES):
        sl = slice(i * bpc, (i + 1) * bpc)
        in_maps.append({
            "x": np.ascontiguousarray(
                x_full[sl].reshape(CELLS_PER_CORE, C_IN)),
            "m": np.ascontiguousarray(
                m_full[sl].reshape(CELLS_PER_CORE)),
            "wd": wd, "ident": ident})
    return in_maps


def kernel(situation, W_size, W_shape, W_rgb, W_agent):
    from concourse.bass_utils import run_bass_kernel_spmd

    key = "prog"
    if key not in _CACHE:
        _CACHE[key] = _build_program(MACROS)
    nc = _CACHE[key]

    in_maps = _in_maps(situation, W_size, W_shape, W_rgb, W_agent)
    res = run_bass_kernel_spmd(nc, in_maps, core_ids=list(range(N_CORES)))
    bpc = B // N_CORES
    out = np.empty((B, H, W, EMB), np.float32)
    for i in range(N_CORES):
        out[i * bpc:(i + 1) * bpc] = res.results[i]["y"].astype(
            np.float32).reshape(bpc, H, W, EMB)
    return out
